# revision 13
# baseline (speedup 1.0000x reference)
"""GNN message-passing layer on 8 TRN2 NeuronCores.

Computes out = relu((adj^T @ x / deg) @ U^T) for N=8192 nodes, D=512 dims.

Sharding: columns of adj (= output rows) are split across the 8 cores;
x and U are replicated, so each core computes a [1024, 512] output slab
with no collectives.

Host-side restaging (pure layout shuffles, no arithmetic): every DRAM
tensor is laid out partition-major so each SBUF partition reads one long
contiguous run; the int32->bf16 cast rides the SWDGE DMA engines.

The per-core DMA bus (~360-385 GB/s) is the binding roofline (53.3 MB
of traffic), so the kernel is one fused j-loop: both 512-column output
halves accumulate simultaneously in all 8 PSUM banks while adj (both
halves) and x stream together - the DMA queues stay saturated for the
whole kernel.

deg is computed entirely off the PE: bf16 per-partition partials on the
DVE (exact: counts <= 64), lane-folded in place, partition-summed by a
gpsimd partition_all_reduce (replicated f32 output), reciprocal on the
DVE; the divide then fuses into PSUM evacuation as an elementwise DVE
multiply, so no PE transposes and no PSUM bank is ever needed for deg.

Ramp/tail shaping: the first and last groups are split into sub-DMAs so
the PE starts ~3us in and the last column-half (h0) finishes only ~2us
after its final adj byte; h1's adj is streamed entirely before h0's
final group, letting h1's deg/evac/output pipeline hide inside h0's
tail streaming window.
"""

import sys

if "/opt/trn_rl_repo" not in sys.path:
    sys.path.insert(0, "/opt/trn_rl_repo")

import numpy as np

import concourse.bacc as bacc
import concourse.bass_isa as bass_isa
import concourse.mybir as mybir
import concourse.tile as tile
from concourse.bass_utils import run_bass_kernel_spmd

N = 8192          # nodes
D = 512           # node dim
NCORES = 8
SH = N // NCORES  # 1024 adj columns (output rows) per core
NJ = N // 128     # 64 contraction tiles
XG = 8            # j-tiles per load group
NG = NJ // XG     # 8 groups
F32 = mybir.dt.float32
BF16 = mybir.dt.bfloat16
I32 = mybir.dt.int32

_compiled = None


def _build():
    nc = bacc.Bacc("TRN2", target_bir_lowering=False, debug=False, num_devices=NCORES)
    # partition-major layouts (see _run for the host-side shuffles)
    x_d = nc.dram_tensor("x", [128, NJ, D], F32, kind="ExternalInput").ap()
    adj_d = nc.dram_tensor("adj", [2, 128, NJ, D], I32, kind="ExternalInput").ap()
    ut_d = nc.dram_tensor("ut", [128, 4, D], F32, kind="ExternalInput").ap()
    out_d = nc.dram_tensor("out", [128, 8, D], F32, kind="ExternalOutput").ap()

    LAST = NG - 1

    with tile.TileContext(nc) as tc:
        with (
            tc.tile_pool(name="xw", bufs=5) as xw_pool,
            tc.tile_pool(name="xs", bufs=3) as xs_pool,
            tc.tile_pool(name="abf", bufs=6) as abf_pool,
            tc.tile_pool(name="cons", bufs=1) as cons_pool,
            tc.tile_pool(name="degp", bufs=1) as degp_pool,
            tc.tile_pool(name="evac", bufs=1) as evac_pool,
            tc.tile_pool(name="osb", bufs=2) as osb_pool,
            tc.tile_pool(name="pacc", bufs=1, space="PSUM") as pacc_pool,
        ):
            ones = cons_pool.tile([128, 128], BF16)
            nc.vector.memset(ones[:], 1.0)

            # 8 PSUM accumulators: [h][c] chunk of aggT
            agg_ps = [
                [
                    pacc_pool.tile([128, D], F32, tag=f"agg{h}{c}", name=f"agg{h}{c}")
                    for c in range(4)
                ]
                for h in range(2)
            ]
            agg_sc = [
                [
                    evac_pool.tile([128, D], BF16, tag=f"aggsc{h}{c}",
                                   name=f"aggsc{h}{c}")
                    for c in range(4)
                ]
                for h in range(2)
            ]
            # per-partition partial degree counts; lane values stay small so
            # bf16 accumulation is exact.  degp holds groups 0..NG-2 (folded
            # in place before the last group); degf holds the last group.
            degp = [
                degp_pool.tile([128, XG, D], BF16, tag=f"degp{h}", name=f"degp{h}")
                for h in range(2)
            ]
            degf = [
                degp_pool.tile([128, 2, D], BF16, tag=f"degf{h}", name=f"degf{h}")
                for h in range(2)
            ]
            for h in range(2):
                nc.vector.memset(degp[h][:], 0.0)
                nc.vector.memset(degf[h][:], 0.0)

            # startup PE filler: ramp the clock / pstate while the first
            # sub-group's DMA lands; garbage lands in agg banks and is reset
            # by the j==0 start=True matmuls
            for f in range(6):
                nc.tensor.matmul(
                    agg_ps[f % 2][(f // 2) % 4][:, 0:128], ones[:], ones[:],
                    start=True, stop=True, skip_group_check=True,
                )

            xg_tiles = [None] * NG
            a_tiles = [[None] * NG for _ in range(2)]

            def load_x(g, subs=1):
                xs = xs_pool.tile([128, XG, D], F32, tag="xs", name=f"xs{g}")
                xg = xw_pool.tile([128, XG, D], BF16, tag="xg", name=f"xg{g}")
                step = XG // subs
                for s in range(subs):
                    lo, hi = s * step, (s + 1) * step
                    nc.sync.dma_start(
                        xs[:, lo:hi, :], x_d[:, g * XG + lo:g * XG + hi, :]
                    )
                    nc.scalar.copy(xg[:, lo:hi, :], xs[:, lo:hi, :])
                xg_tiles[g] = xg

            def load_adj(h, g, subs=1):
                a_bf = abf_pool.tile([128, XG, D], BF16, tag="abf",
                                     name=f"abf{h}_{g}")
                step = XG // subs
                for s in range(subs):
                    lo, hi = s * step, (s + 1) * step
                    nc.gpsimd.dma_start(
                        a_bf[:, lo:hi, :],
                        adj_d[h, :, g * XG + lo:g * XG + hi, :],
                    )
                a_tiles[h][g] = a_bf

            # prime the pipeline: first group fine-grained so the PE starts
            # early; x runs several groups ahead of adj so the cast chain
            # (sync DMA -> ACT cast) never gates the matmul stream
            load_x(0, subs=4)
            load_adj(1, 0, subs=2)
            load_adj(0, 0, subs=2)
            load_x(1, subs=2)
            load_adj(1, 1)
            load_adj(0, 1)
            load_x(2, subs=2)
            load_x(3, subs=2)

            def mm_tile(h, g, t, c):
                j = g * XG + t
                nc.tensor.matmul(
                    agg_ps[h][c][:],
                    xg_tiles[g][:, t, c * 128:(c + 1) * 128],
                    a_tiles[h][g][:, t, :],
                    start=j == 0,
                    stop=j == NJ - 1,
                )

            # ---- steady-state j-loop over groups 0..NG-2 ----
            for g in range(NG - 1):
                if g + 4 < NG:
                    load_x(g + 4, subs=2)
                if g + 2 < NG:
                    if g + 2 == LAST:
                        # last group: h1 entirely before h0, 2-tile sub-DMAs
                        load_adj(1, LAST, subs=4)
                        load_adj(0, LAST, subs=4)
                    else:
                        load_adj(1, g + 2, subs=1)
                        load_adj(0, g + 2, subs=1)
                if g == 4:
                    # U rides the idle HWDGE queue late, cast on ACT
                    u_s = cons_pool.tile([128, 4, D], F32, tag="us", name="us")
                    nc.sync.dma_start(u_s[:], ut_d[:])
                    u_bf = cons_pool.tile([128, 4, D], BF16, tag="ubf", name="ubf")
                    nc.scalar.copy(u_bf[:], u_s[:])
                for h in range(2):
                    nc.vector.tensor_add(degp[h][:], degp[h][:], a_tiles[h][g][:])
                # h1's tile lands fully before h0's (SWDGE queue order), so
                # running all h1 matmuls first hides h0's remaining arrival
                for h in (1, 0):
                    for t in range(XG):
                        for c in range(4):
                            mm_tile(h, g, t, c)

            # in-place lane fold of degp (groups 0..NG-2): 8 -> 1 lanes
            for h in range(2):
                nc.vector.tensor_add(
                    degp[h][:, 0:4, :], degp[h][:, 0:4, :], degp[h][:, 4:8, :]
                )
                nc.vector.tensor_add(
                    degp[h][:, 0:2, :], degp[h][:, 0:2, :], degp[h][:, 2:4, :]
                )
                nc.vector.tensor_add(
                    degp[h][:, 0, :], degp[h][:, 0, :], degp[h][:, 1, :]
                )

            degtot = [None, None]
            recip = [None, None]

            def deg_accum(h):
                # last group's 8 lanes -> degf (2 lanes), + folded degp;
                # everything bf16 (counts <= 64, exact), PAR upcasts to f32
                a = a_tiles[h][LAST]
                for s in range(4):
                    for tt in range(2):
                        nc.vector.tensor_add(
                            degf[h][:, tt, :], degf[h][:, tt, :],
                            a[:, 2 * s + tt, :],
                        )
                ds = evac_pool.tile([128, D], BF16, tag=f"degsum{h}",
                                    name=f"degsum{h}")
                nc.vector.tensor_add(degf[h][:, 0, :], degf[h][:, 0, :],
                                     degf[h][:, 1, :])
                nc.vector.tensor_add(ds[:], degf[h][:, 0, :], degp[h][:, 0, :])
                # partition sum, replicated f32 across all partitions
                dt = evac_pool.tile([128, D], F32, tag=f"degtot{h}",
                                    name=f"degtot{h}")
                nc.gpsimd.partition_all_reduce(
                    dt[:], ds[:], channels=128, reduce_op=bass_isa.ReduceOp.add
                )
                degtot[h] = dt

            def recip_calc(h):
                rc = evac_pool.tile([128, D], F32, tag=f"recip{h}",
                                    name=f"recip{h}")
                nc.vector.reciprocal_approx_fast(rc[:], degtot[h][:])
                recip[h] = rc

            def evac_divide(h, c):
                # agg_sc = agg_ps / deg, fused into the PSUM evacuation
                nc.vector.tensor_mul(agg_sc[h][c][:], agg_ps[h][c][:],
                                     recip[h][:])

            def out_pipeline(h, bank0, bank1, ics):
                for ic in ics:
                    out_ps = pacc_pool.tile([128, D], F32,
                                            tag=(bank0 if ic % 2 == 0 else bank1),
                                            name=f"outps{h}{ic}")
                    for c in range(4):
                        nc.tensor.matmul(
                            out_ps[:],
                            agg_sc[h][c][:, ic * 128:(ic + 1) * 128],
                            u_bf[:, c, :],
                            start=c == 0,
                            stop=c == 3,
                        )
                    out_sb = osb_pool.tile([128, D], F32, tag="osb",
                                           name=f"osb{h}{ic}")
                    nc.scalar.activation(
                        out_sb[:], out_ps[:],
                        mybir.ActivationFunctionType.Relu,
                    )
                    nc.sync.dma_start(out_d[:, h * 4 + ic, :], out_sb[:])

            # ---- final group ----
            # DVE issue order is critical: both deg chains (-> both PARs)
            # must be queued before any recip/evac so neither PAR waits
            # behind a blocked DVE queue head (head-of-line blocking)
            g = LAST
            for s in range(4):
                for tt in range(2):
                    for c in range(4):
                        mm_tile(1, g, 2 * s + tt, c)
            deg_accum(1)
            for s in range(4):
                for tt in range(2):
                    for c in range(4):
                        mm_tile(0, g, 2 * s + tt, c)
            deg_accum(0)
            recip_calc(1)
            for c in range(4):
                evac_divide(1, c)
            recip_calc(0)
            for c in range(4):
                evac_divide(0, c)
            out_pipeline(1, "agg10", "agg11", (0, 1, 2, 3))
            out_pipeline(0, "agg00", "agg01", (0, 1, 2, 3))

    nc.compile()
    return nc


def _get_compiled():
    global _compiled
    if _compiled is None:
        _compiled = _build()
    return _compiled


def _run(x, adj, u, **spmd_kwargs):
    nc = _get_compiled()
    x = np.asarray(x, dtype=np.float32)
    adj = np.asarray(adj, dtype=np.int32)
    u = np.asarray(u, dtype=np.float32)

    # x[t*128+p, d] -> x_r[p, t, d]
    x_r = np.ascontiguousarray(x.reshape(NJ, 128, D).transpose(1, 0, 2))
    # U^T[c*128+p, k] -> ut_r[p, c, k]
    ut_r = np.ascontiguousarray(u.T.reshape(4, 128, D).transpose(1, 0, 2))
    in_maps = []
    for core in range(NCORES):
        shard = adj[:, core * SH:(core + 1) * SH]
        # shard[t*128+p, h*512+d] -> adj_r[h, p, t, d]
        adj_r = np.ascontiguousarray(
            shard.reshape(NJ, 128, 2, D).transpose(2, 1, 0, 3)
        )
        in_maps.append({"x": x_r, "ut": ut_r, "adj": adj_r})

    res = run_bass_kernel_spmd(nc, in_maps, core_ids=list(range(NCORES)), **spmd_kwargs)
    # out_r[p, hic, k] -> out[hic*128+p, k], then stack core slabs
    out = np.concatenate(
        [
            res.results[c]["out"].transpose(1, 0, 2).reshape(SH, D)
            for c in range(NCORES)
        ],
        axis=0,
    )
    return out, res


def kernel(x, adj, U):
    out, _ = _run(x, adj, U)
    return out
